# revision 20
# baseline (speedup 1.0000x reference)
"""Trainium2 Bass kernel for nn_Attention_6743098655482.

Computes, for B=64, H=256, L=8192:
    hidden = concat(sn_hidden, broadcast(mc_hidden))        # [B, 2H, L]
    pre    = tanh(einsum('hk,bkl->bhl', W[0], hidden))      # [B, H, L]
    attns  = einsum('h,bhl->bl', v[0,0], pre)               # [B, L]
    out    = softmax(attns, axis=-1)[:, None, :]            # [B, 1, L]

which is equivalent to (per batch b):
    pre_b  = tanh(W1 @ sn_b + (W2 @ mc_b)[:, None])   W1 = W[0][:, :H], W2 = W[0][:, H:]
    out_b  = softmax(v . pre_b)

Sharding: pure data parallel over batch — 8 batches per core on 8 cores,
small params (W, v) replicated. Per core the kernel streams its 64 MB
sn shard from HBM once (memory-bound regime; ~196us is the achieved
stream floor), runs float32r matmuls (full-rate on the PE), fuses the
+bias and tanh into one ScalarE activation, reduces over H with M=1
matmuls against v, and finishes with a batched softmax over [8, 8192].

Schedule notes (why it is fast):
- The sn stream owns the in-order SP HWDGE queue. Staging-row DMAs are
  issued row_lag units late so their semaphore waits are pre-satisfied
  and never head-of-line-block the queue; the final out-DMAs issue from
  the Act HWDGE queue.
- Each half processes batches 1..7,0: the last unit's batch lives at
  partition 0 (legal compute-write base), so its att copies write attns
  directly and the tail does not wait on a staging-row DMA.
- exp of the first half runs mid-stream (half-outer unit order) with the
  shift-by--||v||_1 trick removing the max dependency; in the loop-timing
  modules the whole softmax tail is software-pipelined across iterations
  (tail_first) with an epilogue after the loop.
"""

import os
import sys

import numpy as np

for _p in ("/opt/trn_rl_repo", "/root/.axon_site/_ro/trn_rl_repo"):
    if os.path.isdir(_p) and _p not in sys.path:
        sys.path.insert(0, _p)

import concourse.bass as bass  # noqa: E402
import concourse.tile as tile  # noqa: E402
from concourse import bacc, mybir  # noqa: E402
from concourse.bass_utils import run_bass_kernel_spmd  # noqa: E402

B, H, L = 64, 256, 8192
NCORES = 8
BL = B // NCORES  # batches per core
F32 = mybir.dt.float32
F32R = mybir.dt.float32r

CH = 1024  # matmul/activation chunk (columns of L)
HDMA = 4096  # columns of L per input DMA

# tunables (overridable before build_module for experiments).
# Defaults are the tuned best: 1024-col chunks, ps_pre [128,1024] x 2 bufs =
# 4 PSUM banks + ps_att [1,512] x 4 bufs for the v-dot matmul outputs.
CFG = {
    "sn_bufs": 7,
    "pre_bufs": 4,
    "ps_pre_bufs": 2,
    "ps_att_bufs": 4,
    "row_bufs": 2,
    "row_lag": 4,  # units between a staging row's data and its SP-queue issue
    "out_on_act": 1,  # issue the final out-DMAs from the Act HWDGE queue
    "tail_first": 1,  # software-pipeline the softmax tail across loop iterations
    "ch": 1024,
    "plan": None,  # per-half chunk sizes; None -> (ch,)*4
    "att_width": 512,  # att psum tile width; default = chunk size
    "att_in_pre": 0,  # host att MM output inside the consumed pre psum tile
    "stream_rowmax": 0,  # per-(b,half) max on DVE during the stream
    "act_copy_frac": 0,  # 1 of every N evacuation copies goes to ScalarE (0=off)
    "first_split": 0,  # split the first half's sn DMAs at chunk boundaries
    "last_plan": None,  # chunk plan override for the final (b, half)
}


def _emit(tc: tile.TileContext, sn, mct, w1t, w2t, vcol, negc, out, reps=1, variant="full", loop_n=None):
    nc = tc.nc
    from contextlib import ExitStack

    with ExitStack() as ctx:
        singles = ctx.enter_context(tc.tile_pool(name="singles", bufs=1))
        sn_pool = ctx.enter_context(tc.tile_pool(name="snp", bufs=CFG["sn_bufs"]))
        pre_pool = ctx.enter_context(tc.tile_pool(name="prep", bufs=CFG["pre_bufs"]))
        ps_pre = ctx.enter_context(tc.tile_pool(name="pspre", bufs=CFG["ps_pre_bufs"], space="PSUM"))
        ps_att = ctx.enter_context(tc.tile_pool(name="psatt", bufs=CFG["ps_att_bufs"], space="PSUM"))

        # --- replicated params -> SBUF ---
        w1_sb = []
        w2_sb = []
        mct_sb = []
        v_sb = []
        for k in range(2):
            w1k = singles.tile([128, H], F32R, tag=f"w1_{k}", name=f"w1_{k}")
            nc.sync.dma_start(out=w1k, in_=w1t[k * 128 : (k + 1) * 128, :])
            w1_sb.append(w1k)
            w2k = singles.tile([128, H], F32R, tag=f"w2_{k}", name=f"w2_{k}")
            nc.sync.dma_start(out=w2k, in_=w2t[k * 128 : (k + 1) * 128, :])
            w2_sb.append(w2k)
            mck = singles.tile([128, BL], F32R, tag=f"mc_{k}", name=f"mc_{k}")
            nc.sync.dma_start(out=mck, in_=mct[k * 128 : (k + 1) * 128, :])
            mct_sb.append(mck)
            vk = singles.tile([128, 1], F32R, tag=f"v_{k}", name=f"v_{k}")
            nc.sync.dma_start(out=vk, in_=vcol[k * 128 : (k + 1) * 128, :])
            v_sb.append(vk)

        # --- bias[m] = W2 @ mc  -> [128 h, BL b] per m-tile ---
        bias_sb = []
        for m in range(2):
            bps = ps_pre.tile([128, BL], F32, tag="pspre", name=f"biasps_{m}")
            for k in range(2):
                nc.tensor.matmul(
                    bps,
                    lhsT=w2_sb[k][:, m * 128 : (m + 1) * 128],
                    rhs=mct_sb[k],
                    start=(k == 0),
                    stop=(k == 1),
                )
            bsb = singles.tile([128, BL], F32, tag=f"bias_{m}", name=f"bias_{m}")
            nc.vector.tensor_copy(out=bsb, in_=bps)
            bias_sb.append(bsb)

        # --- attns accumulator [BL, L] ---
        attns = singles.tile([BL, L], F32, tag="attns", name="attns")
        # compute engines cannot write at partition offsets other than
        # 0/32/64/96, so attns rows b>0 are filled via a partition-0 staging
        # row + SBUF->SBUF DMA
        row_pool = ctx.enter_context(tc.tile_pool(name="rowp", bufs=CFG["row_bufs"]))

        rowmax = singles.tile([1, 2 * BL], F32, tag="rowmax", name="rowmax")
        # softmax shift: exp(x + negc) with host-computed negc = -||v||_1 <= -max
        # (softmax is shift-invariant; this removes the data-max dependency so
        # the first half's exp can run under the tail of the stream)
        negc_sb = singles.tile([BL, 1], F32, tag="negc", name="negc_sb")
        nc.sync.dma_start(out=negc_sb, in_=negc)
        sums2 = singles.tile([BL, 2], F32, tag="sums2", name="sums2")
        def emit_tail(suffix):
            # --- softmax over L, batched across the 8 local batches ---
            nc.scalar.activation(
                out=attns[:, L // 2 : L],
                in_=attns[:, L // 2 : L],
                func=mybir.ActivationFunctionType.Exp,
                bias=negc_sb,
                accum_out=sums2[:, 1:2],
            )
            sums = singles.tile([BL, 1], F32, tag="sums", name=f"sums_{suffix}")
            nc.vector.reduce_sum(
                out=sums, in_=sums2, axis=mybir.AxisListType.X
            )
            rec = singles.tile([BL, 1], F32, tag="rec", name=f"rec_{suffix}")
            nc.vector.reciprocal(out=rec, in_=sums)
            # scale the two halves on different engines (DVE 2x perf mode does
            # h0 in ~2.2us while Act's Copy does h1). Out-DMAs go on the Act
            # HWDGE queue — NOT SP — so the next loop iteration's sn stream
            # isn't queued behind them; Act op order (copy, dma h1, dma h0)
            # keeps Act's own queue from blocking on the DVE half.
            nc.vector.tensor_scalar_mul(
                out=attns[:, 0 : L // 2], in0=attns[:, 0 : L // 2], scalar1=rec
            )
            nc.scalar.activation(
                out=attns[:, L // 2 : L],
                in_=attns[:, L // 2 : L],
                func=mybir.ActivationFunctionType.Copy,
                scale=rec,
            )
            eng = nc.scalar if CFG["out_on_act"] else nc.sync
            eng.dma_start(out=out[:, L // 2 : L], in_=attns[:, L // 2 : L])
            eng.dma_start(out=out[:, 0 : L // 2], in_=attns[:, 0 : L // 2])


        if loop_n is not None:
            loop_cm = tc.For_i(
                0,
                loop_n,
                1,
                hint_engines=(
                    mybir.EngineType.PE,
                    mybir.EngineType.Activation,
                    mybir.EngineType.DVE,
                    mybir.EngineType.SP,
                ),
            )
            loop_cm.__enter__()
        for rep in range(reps):
            if variant == "full" and loop_n is not None and CFG["tail_first"]:
                # software-pipelined: this body's tail processes the PREVIOUS
                # iteration's attns while this iteration's sn stream runs.
                # Iteration 0 pushes garbage through (overwritten by later
                # iterations); the final iteration's tail is the epilogue.
                emit_tail(f"b{rep}")
            # --- main stream over batches ---
            # The att stage (v-dot matmuls + PSUM evacuation) is emitted with a
            # one-chunk lag so the in-order PE never stalls waiting on the
            # activation of the current chunk: ... pre(c) att(c-1) pre(c+1) ...
            #
            # Batch order is 1..7,0 within each half: the last unit's batch
            # sits at partition 0 — a legal compute-write base — so its att
            # copies land in attns directly (no staging row DMA on the tail).
            #
            # Row DMAs (partition-0 staging row -> attns[b]) are issued on the
            # in-order SP queue, which HOLDS the queue while the DMA's wait is
            # pending. Issuing row(U) right after unit U would cap the sn
            # stream's lookahead at U's compute latency, so row(U) is emitted
            # after unit U+ROW_LAG's sn DMAs — by then its data is ready and
            # the issue costs ~0.6us with no head-of-line block.
            pending = [None]
            chunk_ctr = [0]
            copy_ctr = [0]
            pending_rows = []  # (unit_idx, emit_fn)
            unit_ctr = [0]
            ROW_LAG = CFG["row_lag"]

            def flush_pending():
                if pending[0] is not None:
                    pending[0]()
                    pending[0] = None

            def flush_rows(upto):
                while pending_rows and pending_rows[0][0] <= upto:
                    pending_rows.pop(0)[1]()

            border = list(range(1, BL)) + [0]
            for half in range(2):
                for b in border:
                    last_unit = b == 0
                    if CFG["plan"]:
                        plan = list(CFG["plan"])
                        assert sum(plan) == HDMA
                    else:
                        plan = [CFG["ch"]] * (HDMA // CFG["ch"])
                    if CFG["last_plan"] and last_unit and half == 1:
                        plan = list(CFG["last_plan"])
                        assert sum(plan) == HDMA
                    row = (
                        None
                        if last_unit
                        else row_pool.tile(
                            [1, HDMA], F32, tag="row", name=f"row_{rep}_{b}_{half}"
                        )
                    )
                    snt = [
                        sn_pool.tile([128, HDMA], F32R, tag="sn", name=f"sn_{rep}_{b}_{half}_{k}")
                        for k in range(2)
                    ]
                    for k in range(2):
                        nc.sync.dma_start(
                            out=snt[k],
                            in_=sn[b, k * 128 : (k + 1) * 128, half * HDMA : (half + 1) * HDMA],
                        )
                    flush_rows(unit_ctr[0] - ROW_LAG)
                    unit_ctr[0] += 1
                    if variant == "dma_only":
                        continue
                    n_chunks = len(plan)
                    offs = [sum(plan[:i]) for i in range(n_chunks)]
                    for c in range(n_chunks):
                        col0 = offs[c]
                        CHV = plan[c]
                        pre_sbs = []
                        pps_list = []
                        for m in range(2):
                            pps = ps_pre.tile([128, CHV], F32, tag="pspre", name=f"pps_{rep}_{b}_{half}_{c}_{m}")
                            pps_list.append(pps)
                            for s in range(CHV // 512):
                                for k in range(2):
                                    nc.tensor.matmul(
                                        pps[:, s * 512 : (s + 1) * 512],
                                        lhsT=w1_sb[k][:, m * 128 : (m + 1) * 128],
                                        rhs=snt[k][:, col0 + s * 512 : col0 + (s + 1) * 512],
                                        start=(k == 0),
                                        stop=(k == 1),
                                    )
                            if variant == "mm_only":
                                continue
                            psb = pre_pool.tile([128, CHV], F32R, tag="pre", name=f"pre_{rep}_{b}_{half}_{c}_{m}")
                            nc.scalar.activation(
                                out=psb,
                                in_=pps,
                                func=mybir.ActivationFunctionType.Tanh,
                                bias=bias_sb[m][:, b : b + 1],
                            )
                            pre_sbs.append(psb)
                        if variant in ("mm_only", "pre_only"):
                            continue
                        flush_pending()

                        chunk_ctr[0] += 1

                        def att_stage(
                            rep=rep, b=b, half=half, c=c, col0=col0,
                            row=row, pre_sbs=pre_sbs, CHV=CHV, n_chunks=n_chunks,
                            pps_list=pps_list, parity=chunk_ctr[0] % 2,
                            last_unit=last_unit, unit=unit_ctr[0] - 1,
                        ):
                            # the last unit's batch is partition 0: copies go
                            # straight into attns, skipping the staging row
                            if last_unit:
                                dst, dcol = attns[0:1, :], half * HDMA + col0
                            else:
                                dst, dcol = row, col0
                            if CFG["att_in_pre"]:
                                host = pps_list[parity]
                                for s in range(CHV // 512):
                                    for m in range(2):
                                        nc.tensor.matmul(
                                            host[0:1, s * 512 : (s + 1) * 512],
                                            lhsT=v_sb[m],
                                            rhs=pre_sbs[m][:, s * 512 : (s + 1) * 512],
                                            start=(m == 0),
                                            stop=(m == 1),
                                            skip_group_check=True,
                                        )
                                nc.vector.tensor_copy(
                                    out=dst[0:1, dcol : dcol + CHV],
                                    in_=host[0:1, 0:CHV],
                                )
                            else:
                                aw = CFG["att_width"] or CHV
                                for a0 in range(0, CHV, aw):
                                    w = min(aw, CHV - a0)
                                    aps = ps_att.tile([1, w], F32, tag="att", name=f"att_{rep}_{b}_{half}_{c}_{a0}")
                                    for s in range(w // 512):
                                        for m in range(2):
                                            nc.tensor.matmul(
                                                aps[:, s * 512 : (s + 1) * 512],
                                                lhsT=v_sb[m],
                                                rhs=pre_sbs[m][:, a0 + s * 512 : a0 + (s + 1) * 512],
                                                start=(m == 0),
                                                stop=(m == 1),
                                            )
                                    copy_ctr[0] += 1
                                    nc.vector.tensor_copy(
                                        out=dst[0:1, dcol + a0 : dcol + a0 + w],
                                        in_=aps,
                                    )
                            if c == n_chunks - 1:
                                if not last_unit and variant in ("full", "no_tail"):
                                    def emit_row(b=b, half=half, row=row):
                                        nc.sync.dma_start(
                                            out=attns[b : b + 1, half * HDMA : (half + 1) * HDMA],
                                            in_=row,
                                        )
                                    pending_rows.append((unit, emit_row))
                                if variant == "full" and last_unit and half == 0:
                                    # all batches' first halves are complete:
                                    # exp+partial-sum of attns[:, :L//2] runs
                                    # under the second-half stream. Deferred
                                    # through pending_rows (FIFO) so it is
                                    # emitted after every half-0 row DMA at
                                    # any row_lag.
                                    def emit_exp0():
                                        nc.scalar.activation(
                                            out=attns[:, 0 : L // 2],
                                            in_=attns[:, 0 : L // 2],
                                            func=mybir.ActivationFunctionType.Exp,
                                            bias=negc_sb,
                                            accum_out=sums2[:, 0:1],
                                        )
                                    pending_rows.append((unit, emit_exp0))

                        pending[0] = att_stage
            flush_pending()
            flush_rows(10**9)

        if variant == "full" and not (loop_n is not None and CFG["tail_first"]):
            emit_tail("t")
        if loop_n is not None:
            loop_cm.__exit__(None, None, None)
            if variant == "full" and CFG["tail_first"]:
                # epilogue: the software-pipelined tail of the final iteration
                emit_tail("ep")


def build_module(reps=1, variant="full", loop_n=None):
    nc = bacc.Bacc(
        "TRN2",
        debug=False,
        enable_asserts=False,
        target_bir_lowering=False,
    )
    sn = nc.dram_tensor("sn", [BL, H, L], F32R, kind="ExternalInput").ap()
    mct = nc.dram_tensor("mct", [H, BL], F32R, kind="ExternalInput").ap()
    w1t = nc.dram_tensor("w1t", [H, H], F32R, kind="ExternalInput").ap()
    w2t = nc.dram_tensor("w2t", [H, H], F32R, kind="ExternalInput").ap()
    vcol = nc.dram_tensor("vcol", [H, 1], F32R, kind="ExternalInput").ap()
    negc = nc.dram_tensor("negc", [BL, 1], F32, kind="ExternalInput").ap()
    out = nc.dram_tensor("out", [BL, L], F32, kind="ExternalOutput").ap()
    with tile.TileContext(nc) as tc:
        _emit(tc, sn, mct, w1t, w2t, vcol, negc, out, reps=reps, variant=variant, loop_n=loop_n)
    nc.compile()
    return nc


_NC = None


def _get_module():
    global _NC
    if _NC is None:
        _NC = build_module()
    return _NC


def make_in_maps(mc_hidden, sn_hidden, v, W):
    """Shard FULL inputs into per-core in_maps (host-side, cheap)."""
    w0 = np.asarray(W, dtype=np.float32)[0]  # [H, 2H]
    w1t = np.ascontiguousarray(w0[:, :H].T)  # [H(k), H(h)]
    w2t = np.ascontiguousarray(w0[:, H:].T)  # [H(k), H(h)]
    vcol = np.ascontiguousarray(np.asarray(v, dtype=np.float32)[0, 0][:, None])
    # upper bound on |attns| = |v . tanh(...)| <= ||v||_1; softmax is invariant
    # to the shift and exp(x - c) stays in fp32 range
    negc = np.full((BL, 1), -np.abs(vcol).sum(), dtype=np.float32)
    mc = np.asarray(mc_hidden, dtype=np.float32)
    sn = np.asarray(sn_hidden, dtype=np.float32)
    in_maps = []
    for c in range(NCORES):
        sl = slice(c * BL, (c + 1) * BL)
        in_maps.append(
            {
                "sn": np.ascontiguousarray(sn[sl]),
                "mct": np.ascontiguousarray(mc[sl].T),
                "w1t": w1t,
                "w2t": w2t,
                "vcol": vcol,
                "negc": negc,
            }
        )
    return in_maps


def run(mc_hidden, sn_hidden, v, W, trace=False):
    nc = _get_module()
    in_maps = make_in_maps(mc_hidden, sn_hidden, v, W)
    # NTFF tracing is unavailable under this axon build (antenv.axon_hooks
    # missing) — force the non-traced PJRT path.
    res = run_bass_kernel_spmd(nc, in_maps, core_ids=list(range(NCORES)), trace=False)
    full = np.concatenate([np.asarray(r["out"]) for r in res.results], axis=0)
    return full[:, None, :].astype(np.float32), res


def kernel(mc_hidden, sn_hidden, v, W):
    out, _ = run(mc_hidden, sn_hidden, v, W, trace=False)
    return out

